# revision 1
# baseline (speedup 1.0000x reference)
"""Multi-head attention (B=2, T=2048, D=1024, H=16) on 8 Trainium2 NeuronCores.

Sharding: tensor-parallel over heads — core c owns global heads {2c, 2c+1} for
both batch elements (Wq/Wk/Wv column-split, Wo row-split, relpos_bias split
along H).  Each core computes a partial [B, D, T] output-projection product;
the host sums the 8 partials and transposes back to [B, T, D].  SPMD: one
program, per-core weight/relpos slices in the input maps; no collectives.

Device-side layout ("transposed flash attention"): scores are computed as
S^T[k, q] so the exp'd scores are already in the right layout (k on
partitions) to be the moving operand of the P@V matmul — the attention
matrix is never transposed on device.

Design notes:
  - fp16 matmuls everywhere (1 cyc/col on PE, ~8x more mantissa than bf16):
    final absmax error ~2e-3 (rel ~6e-4) vs the fp32 reference.
  - relpos bias is added into the scores PSUM accumulation by an
    identity-stationary fp8 matmul (a DVE tensor_tensor add would run at
    1x mode and dominate).  The causal mask is baked into relposT on the
    host as -240 (exp(S-240) underflows to exactly 0; fp8e4 can't carry
    -1e30).  Fully-masked k-blocks are skipped and diagonal-band blocks are
    column-restricted to the causal wavefront (~38% less attention work).
  - key-pad mask rides the ACT exp instruction as a per-partition bias.
  - softmax max-subtraction is skipped (scores are O(+-10), exp is safe in
    fp32); the denominator comes free as an extra row of the P@V matmul
    from an all-ones column appended to V; 1/sqrt(dk) is folded into Wq.
  - normalization (1/denom broadcast along partitions) uses
    reciprocal_approx_fast (SBUF source only — it silently corrupts from
    PSUM, and single-partition slices at base 64 return zeros) and an
    exact hi/lo-fp16 ones-outer-product matmul pair.
  - the whole program is emitted as one software-pipelined stream: batch-1
    projections, per-q-group normalizations, and all output-projection
    pieces are interleaved into the attention k-loops so the PE never
    idles long enough to re-throttle (HAM) and DMA stays behind compute.
  - weight-stationary emission order (all S, all relpos-add, all P@V per
    k-chunk) keeps same-weight matmuls back-to-back.
"""

import sys

for p in ("/opt/trn_rl_repo", "/root/.axon_site/_ro/trn_rl_repo"):
    if p not in sys.path:
        sys.path.insert(0, p)

import numpy as np
import ml_dtypes

import concourse.bacc as bacc
import concourse.mybir as mybir
import concourse.tile as tile
from concourse.bass_utils import run_bass_kernel_spmd

B, T, D, H = 2, 2048, 1024, 16
DK = D // H          # 64
NCORES = 8
HPC = H // NCORES    # heads per core = 2
QG = 512             # q-group width
NQG = T // QG        # 4
NKC = T // 128       # 16 k-chunks
NDC = D // 128       # 8 d-chunks
NEG = np.float32(-1e30)

F32 = mybir.dt.float32
FP16 = mybir.dt.float16
FP8 = mybir.dt.float8e4

_CACHE = {}


def _build_program():
    nc = bacc.Bacc("TRN2", target_bir_lowering=False, debug=False,
                   enable_asserts=True)

    d_qT = nc.dram_tensor("qT", [B, D, T], FP16, kind="ExternalInput").ap()
    d_kT = nc.dram_tensor("kT", [B, D, T], FP16, kind="ExternalInput").ap()
    d_vT = nc.dram_tensor("vT", [B, D, T], FP16, kind="ExternalInput").ap()
    d_rp = nc.dram_tensor("relposT", [HPC, T, T], FP8, kind="ExternalInput").ap()
    d_kp = nc.dram_tensor("kpadT", [128, B, NKC], F32, kind="ExternalInput").ap()
    d_wq = nc.dram_tensor("wqT", [D, 128], FP16, kind="ExternalInput").ap()
    d_wk = nc.dram_tensor("wkT", [D, 128], FP16, kind="ExternalInput").ap()
    d_wv = nc.dram_tensor("wvT", [D, 128], FP16, kind="ExternalInput").ap()
    d_wo = nc.dram_tensor("woT", [HPC, DK, D], FP16, kind="ExternalInput").ap()
    d_id8 = nc.dram_tensor("id8", [128, 128], FP8, kind="ExternalInput").ap()
    d_out = nc.dram_tensor("outT", [B, D, T], FP16, kind="ExternalOutput").ap()

    with tile.TileContext(nc) as tc:
        with (
            tc.tile_pool(name="persist", bufs=1) as persist,
            tc.tile_pool(name="stream", bufs=6) as stream,
            tc.tile_pool(name="rp", bufs=6) as rppool,
            tc.tile_pool(name="ee", bufs=5) as epool,
            tc.tile_pool(name="oc", bufs=3) as ocpool,
            tc.tile_pool(name="nrm", bufs=2) as nrm,
            tc.tile_pool(name="ps", bufs=4, space="PSUM") as ps,
            tc.tile_pool(name="opsum", bufs=4, space="PSUM") as ops,
        ):
            # ---- constants ----
            id8 = persist.tile([128, 128], FP8, tag="id8", name="id8")
            nc.scalar.dma_start(out=id8[:], in_=d_id8[:])
            kpad = persist.tile([128, B, NKC], F32, tag="kpad", name="kpad")
            nc.scalar.dma_start(out=kpad[:], in_=d_kp[:])
            ones = persist.tile([128, DK], F32, tag="ones", name="ones")
            nc.vector.memset(ones[:], 1.0)
            ones16 = persist.tile([128, DK], FP16, tag="ones16", name="ones16")
            nc.vector.memset(ones16[:], 1.0)

            w_sb = {}
            for nm, dten in (("q", d_wq), ("k", d_wk), ("v", d_wv)):
                w = persist.tile([128, NDC, 128], FP16, tag=f"w{nm}",
                                 name=f"w{nm}")
                nc.scalar.dma_start(
                    out=w[:], in_=dten.rearrange("(a p) m -> p a m", p=128))
                w_sb[nm] = w
            wo_sb = []
            for h in range(HPC):
                w = persist.tile([DK, D], FP16, tag=f"wo{h}", name=f"wo{h}")
                nc.scalar.dma_start(out=w[:], in_=d_wo[h])
                wo_sb.append(w)

            qt_sb, kt_sb = {}, {}
            vaug = {}
            for b in range(B):
                qt_sb[b] = persist.tile([128, T], FP16, tag=f"qt{b}",
                                        name=f"qt{b}")
                kt_sb[b] = persist.tile([128, T], FP16, tag=f"kt{b}",
                                        name=f"kt{b}")
                for h in range(HPC):
                    va = persist.tile([128, NKC * 80], FP16, tag=f"va{b}{h}",
                                      name=f"va{b}{h}")
                    va_c = va[:].rearrange("p (c u) -> p c u", u=80)
                    nc.vector.tensor_copy(va_c[:, :, 64], ones[:, 0:NKC])
                    vaug[(b, h)] = va

            # ---- projection helpers (weight-stationary, dk outer) ----
            def load_x(dten, b, dk, eng, tag="xin", bufs=10):
                t = stream.tile([128, T], FP16, tag=tag, bufs=bufs,
                                name=f"x{tag}{b}{dk}")
                eng.dma_start(out=t[:],
                              in_=dten[b, dk * 128:(dk + 1) * 128, :])
                return t

            def proj_qk(nm, b, xts, dst):
                accs = [ps.tile([128, QG], F32, tag="ps", name="ps")
                        for _ in range(NQG)]
                for dk in range(NDC):
                    for cc in range(NQG):
                        nc.tensor.matmul(
                            accs[cc][:], w_sb[nm][:, dk, :],
                            xts[dk][:, cc * QG:(cc + 1) * QG],
                            start=(dk == 0), stop=(dk == NDC - 1))
                for cc in range(NQG):
                    nc.vector.tensor_copy(
                        dst[b][:, cc * QG:(cc + 1) * QG], accs[cc][:])

            def proj_v(b, vts):
                for tb in range(NKC):
                    ts_ = slice(tb * 128, (tb + 1) * 128)
                    acc = ps.tile([128, 128], F32, tag="ps", name="psv")
                    for dk in range(NDC):
                        nc.tensor.matmul(
                            acc[:], vts[dk][:, ts_], w_sb["v"][:, dk, :],
                            start=(dk == 0), stop=(dk == NDC - 1))
                    for h in range(HPC):
                        nc.vector.tensor_copy(
                            vaug[(b, h)][:, tb * 80:tb * 80 + DK],
                            acc[:, h * DK:(h + 1) * DK])

            # ---- phase 1: batch 0 projections; loads split across rings ----
            engs = (nc.scalar, nc.sync)
            xq0 = {dk: load_x(d_qT, 0, dk, engs[dk % 2]) for dk in range(NDC)}
            vch0 = {dk: load_x(d_vT, 0, dk, engs[dk % 2], "xinv", 8)
                    for dk in range(NDC)}
            proj_qk("q", 0, xq0, qt_sb)
            xk0 = {dk: load_x(d_kT, 0, dk, engs[dk % 2]) for dk in range(NDC)}
            proj_v(0, vch0)
            proj_qk("k", 0, xk0, kt_sb)

            # ---- phase 2: attention; batch-1 projections and all output
            # projections are interleaved into the instruction stream ----
            at_sb = {}
            for b in range(B):
                for h in range(HPC):
                    at_sb[(b, h)] = persist.tile([DK, T], FP16,
                                                 tag=f"at{b}{h}",
                                                 name=f"at{b}{h}")

            def norm_dve(u, qg):
                o = nrm.tile([DK + 1, QG], F32, tag="oc2", name="oc2")
                nc.vector.tensor_copy(o[:], pend_ops[u][qg][:])
                rc = nrm.tile([DK + 1, QG], F32, tag="rc", name="rc")
                nc.vector.reciprocal_approx_fast(out=rc[:], in_=o[:])
                rch = nrm.tile([DK + 1, QG], FP16, tag="rch", name="rch")
                nc.vector.tensor_copy(rch[:], rc[:])
                rcl = nrm.tile([DK + 1, QG], FP16, tag="rcl", name="rcl")
                nc.vector.tensor_sub(rcl[:], rc[:], rch[:])
                pend_dve[(u, qg)] = (o, rch, rcl)

            def norm_pe(u, qg):
                b, h = u
                o, rch, rcl = pend_dve.pop((u, qg))
                rb = ops.tile([DK, QG], F32, tag="ops", name="rb")
                nc.tensor.matmul(
                    rb[:], ones16[DK:DK + 1, :], rch[DK:DK + 1, :],
                    start=True, stop=False)
                nc.tensor.matmul(
                    rb[:], ones16[DK:DK + 1, :], rcl[DK:DK + 1, :],
                    start=False, stop=True)
                nc.vector.tensor_mul(
                    at_sb[(b, h)][:, qg * QG:(qg + 1) * QG], o[0:DK, :], rb[:])

            def oproj_piece(b, db, qg):
                ds_ = slice(db * 128, (db + 1) * 128)
                qs = slice(qg * QG, (qg + 1) * QG)
                pp = ps.tile([128, QG], F32, tag="ps", name="pp")
                for h in range(HPC):
                    nc.tensor.matmul(
                        pp[:], wo_sb[h][:, ds_], at_sb[(b, h)][:, qs],
                        start=(h == 0), stop=(h == HPC - 1))
                oc = ocpool.tile([128, QG], FP16, tag="oc", name="oc")
                nc.vector.tensor_copy(oc[:], pp[:])
                nc.sync.dma_start(out=d_out[b, ds_, qs], in_=oc[:])

            pend_ops, pend_dve = {}, {}
            pending = []          # (unit, qg) whose norm_pe is still owed
            oproj_q = []          # (b, db, qg) out-proj pieces ready to emit
            units = [(b, h) for b in range(B) for h in range(HPC)]
            last_u = units[-1]
            xq1 = vch1 = None
            for ui, u in enumerate(units):
                b, h = u
                hs = slice(h * DK, (h + 1) * DK)
                if ui == 0:
                    # prefetch batch-1 q and v rows.  These issues can wait
                    # minutes-long (in engine terms) on pool-slot semaphores,
                    # so they MUST NOT sit in the Scalar queue where they
                    # would stall the exp instructions behind them; the
                    # otherwise-idle GpSimd (SWDGE) queue absorbs the waits.
                    xq1 = {dk: load_x(d_qT, 1, dk, nc.gpsimd)
                           for dk in range(NDC)}
                    vch1 = {dk: load_x(d_vT, 1, dk, nc.gpsimd, "xinv", 8)
                            for dk in range(NDC)}
                if ui == 1:
                    # batch-1 projections: data already resident
                    proj_qk("q", 1, xq1, qt_sb)
                    xk1 = {dk: load_x(d_kT, 1, dk, nc.gpsimd)
                           for dk in range(NDC)}
                    proj_v(1, vch1)
                    proj_qk("k", 1, xk1, kt_sb)
                pend_ops[u] = [ops.tile([DK + 1, QG], F32, tag="ops",
                                        name="ops") for _ in range(NQG)]
                o_ps = pend_ops[u]
                for ck in range(NKC):
                    if ck == 4 and pending:
                        norm_pe(*pending.pop(0))
                    if ck >= 5 and (ck - 5) % 4 == 0:
                        qgn = (ck - 5) // 4
                        norm_pe(u, qgn)
                        if u == last_u and qgn < NQG - 1:
                            oproj_q.extend((1, db, qgn) for db in range(NDC))
                    if ck >= 4 and oproj_q:
                        oproj_piece(*oproj_q.pop(0))
                        if len(oproj_q) > 24 - (NKC - ck):
                            oproj_piece(*oproj_q.pop(0))
                    qg0 = ck // 4          # first valid q-group
                    off = qg0 * QG         # start col of rp tile
                    rp_t = rppool.tile([128, T], FP8, tag="rp", name="rp")
                    nc.sync.dma_start(
                        out=rp_t[:, 0:T - off],
                        in_=d_rp[h, ck * 128:(ck + 1) * 128, off:T])
                    s_ts, cos, ws = [], [], []
                    for qg in range(qg0, NQG):
                        co = max(0, ck * 128 - qg * QG)
                        w = QG - co
                        cos.append(co)
                        ws.append(w)
                        s_t = ps.tile([128, QG], F32, tag="ps", name="ps")
                        s_ts.append(s_t)
                        nc.tensor.matmul(
                            s_t[:, 0:w],
                            kt_sb[b][hs, ck * 128:(ck + 1) * 128],
                            qt_sb[b][hs, qg * QG + co:(qg + 1) * QG],
                            start=True, stop=False)
                    for i, qg in enumerate(range(qg0, NQG)):
                        rj = qg * QG + cos[i] - off
                        nc.tensor.matmul(
                            s_ts[i][:, 0:ws[i]], id8[:],
                            rp_t[:, rj:rj + ws[i]],
                            start=False, stop=True)
                    e_ts = []
                    for i, qg in enumerate(range(qg0, NQG)):
                        e_t = epool.tile([128, QG], FP16, tag="ee", name="ee")
                        e_ts.append(e_t)
                        nc.scalar.activation(
                            e_t[:, 0:ws[i]], s_ts[i][:, 0:ws[i]],
                            mybir.ActivationFunctionType.Exp,
                            bias=kpad[:, b, ck:ck + 1])
                    for i, qg in enumerate(range(qg0, NQG)):
                        nc.tensor.matmul(
                            o_ps[qg][:, cos[i]:QG],
                            vaug[(b, h)][:, ck * 80:ck * 80 + DK + 1],
                            e_ts[i][:, 0:ws[i]],
                            start=(ck == 0), stop=(ck == 4 * qg + 3))
                    if ck % 4 == 3:
                        norm_dve(u, ck // 4)
                pending.append((u, NQG - 1))
                if u == (0, HPC - 1):
                    # batch 0 fully attended: queue its out-projection
                    # (qg-major so the still-pending qg3 pieces come last;
                    # the pending norm_pe pops at the next unit's ck=4,
                    # before any qg3 piece is emitted)
                    oproj_q.extend((0, db, qg)
                                   for qg in range(NQG) for db in range(NDC))
            while pending:
                norm_pe(*pending.pop(0))
            oproj_q.extend((1, db, NQG - 1) for db in range(NDC))
            while oproj_q:
                oproj_piece(*oproj_q.pop(0))

    nc.compile()
    return nc


def _prep_host(q, k, v, key_pad_mask, attn_mask, relpos_bias, Wq, Wk, Wv, Wo):
    f32, f16 = np.float32, np.float16
    qT = np.asarray(q, f32).transpose(0, 2, 1).astype(f16)
    kT = np.asarray(k, f32).transpose(0, 2, 1).astype(f16)
    vT = np.asarray(v, f32).transpose(0, 2, 1).astype(f16)

    kb = np.where(np.asarray(key_pad_mask), NEG, f32(0)).astype(f32)  # [B,T]
    kpadT = np.ascontiguousarray(kb.reshape(B, NKC, 128).transpose(2, 0, 1))

    maskT = np.asarray(attn_mask).T  # [k, q], True = masked (k > q)
    rp = np.asarray(relpos_bias, f32)

    id8 = np.eye(128, dtype=ml_dtypes.float8_e4m3)

    Wq = np.asarray(Wq, f32) * f32(1.0 / np.sqrt(DK))
    Wk = np.asarray(Wk, f32)
    Wv = np.asarray(Wv, f32)
    Wo = np.asarray(Wo, f32)

    in_maps = []
    for c in range(NCORES):
        rows = slice(c * 128, (c + 1) * 128)
        h0 = 2 * c
        rpT = np.where(maskT[None], f32(-240.0),
                       rp[h0:h0 + 2].transpose(0, 2, 1)).astype(
                           ml_dtypes.float8_e4m3)
        woT = np.ascontiguousarray(
            np.stack([Wo[:, (2 * c + h) * DK:(2 * c + h + 1) * DK].T
                      for h in range(HPC)])).astype(f16)
        in_maps.append({
            "qT": qT, "kT": kT, "vT": vT,
            "relposT": np.ascontiguousarray(rpT),
            "kpadT": kpadT,
            "wqT": np.ascontiguousarray(Wq[rows].T).astype(f16),
            "wkT": np.ascontiguousarray(Wk[rows].T).astype(f16),
            "wvT": np.ascontiguousarray(Wv[rows].T).astype(f16),
            "woT": woT,
            "id8": id8,
        })
    return in_maps


def run(trace=False, tmpdir=None, **inputs):
    if "nc" not in _CACHE:
        _CACHE["nc"] = _build_program()
    nc = _CACHE["nc"]
    in_maps = _prep_host(**inputs)
    res = run_bass_kernel_spmd(nc, in_maps, core_ids=list(range(NCORES)),
                               trace=trace, tmpdir=tmpdir)
    acc = res.results[0]["outT"].astype(np.float64)
    for c in range(1, NCORES):
        acc += res.results[c]["outT"]
    out = np.ascontiguousarray(acc.transpose(0, 2, 1)).astype(np.float32)
    return out, res


def kernel(**inputs) -> np.ndarray:
    out, _ = run(trace=False, **inputs)
    return out



# revision 2
# speedup vs baseline: 1.1869x; 1.1869x over previous
"""Multi-head attention (B=2, T=2048, D=1024, H=16) on 8 Trainium2 NeuronCores.

Sharding: tensor-parallel over heads — core c owns global heads {2c, 2c+1} for
both batch elements (Wq/Wk/Wv column-split, Wo row-split, relpos_bias split
along H).  Each core computes a partial [B, D, T] output-projection product;
the host sums the 8 partials and transposes back to [B, T, D].  SPMD: one
program, per-core weight/relpos slices in the input maps; no collectives.

Device-side layout ("transposed flash attention"): scores are computed as
S^T[k, q] so the exp'd scores are already in the right layout (k on
partitions) to be the moving operand of the P@V matmul — the attention
matrix is never transposed on device.

v2 changes over the 272us baseline (all aimed at PE column count, the
measured bottleneck: PE runs at 2.4GHz with LDWEIGHTS fully pipelined):
  - key-pad chunk skip: trailing fully-masked k-chunks (lengths are
    data-dependent; program is compiled per active-chunk-count tuple and
    cached) are skipped in scores/relpos/exp/PV, and the k/v loads +
    projections are column-trimmed to the active range.
  - head-packed output projection: the two heads' normalized attention
    outputs live stacked in one [128, T] tile (h0 on partitions 0-63, h1 on
    64-127 via a small SBUF->SBUF DMA partition move), so each oproj piece
    is ONE 128-contraction matmul instead of two 64-contraction ones.
  - relpos tiles are loaded once per head and kept resident in SBUF,
    shared by both batches (was: loaded twice).
  - normalization broadcast uses a single fp16 ones-outer-product matmul
    (the fp16 lo-term refinement was dropped; ~2.4e-4 extra rel err).
  - oproj pieces are queued as soon as both heads' norms for a (b, qg) are
    emitted and paced into every later k-loop iteration; oc casts alternate
    DVE/ACT so the tail is not serialized on one engine.
"""

import sys

for p in ("/opt/trn_rl_repo", "/root/.axon_site/_ro/trn_rl_repo"):
    if p not in sys.path:
        sys.path.insert(0, p)

import numpy as np
import ml_dtypes

import concourse.bacc as bacc
import concourse.mybir as mybir
import concourse.tile as tile
from concourse.bass_utils import run_bass_kernel_spmd

B, T, D, H = 2, 2048, 1024, 16
DK = D // H          # 64
NCORES = 8
HPC = H // NCORES    # heads per core = 2
QG = 512             # q-group width
NQG = T // QG        # 4
NKC = T // 128       # 16 k-chunks max
NDC = D // 128       # 8 d-chunks
NEG = np.float32(-1e30)

F32 = mybir.dt.float32
FP16 = mybir.dt.float16
FP8 = mybir.dt.float8e4

_CACHE = {}


def _build_program(nkcb):
    nkc_max = max(nkcb)
    nc = bacc.Bacc("TRN2", target_bir_lowering=False, debug=False,
                   enable_asserts=True)

    d_qT = nc.dram_tensor("qT", [B, D, T], FP16, kind="ExternalInput").ap()
    d_kT = nc.dram_tensor("kT", [B, D, T], FP16, kind="ExternalInput").ap()
    d_vT = nc.dram_tensor("vT", [B, D, T], FP16, kind="ExternalInput").ap()
    d_rp = nc.dram_tensor("relposT", [HPC, T, T], FP8, kind="ExternalInput").ap()
    d_kp = nc.dram_tensor("kpadT", [128, B, NKC], F32, kind="ExternalInput").ap()
    d_wq = nc.dram_tensor("wqT", [D, 128], FP16, kind="ExternalInput").ap()
    d_wk = nc.dram_tensor("wkT", [D, 128], FP16, kind="ExternalInput").ap()
    d_wv = nc.dram_tensor("wvT", [D, 128], FP16, kind="ExternalInput").ap()
    d_wo = nc.dram_tensor("woT", [128, D], FP16, kind="ExternalInput").ap()
    d_id8 = nc.dram_tensor("id8", [128, 128], FP8, kind="ExternalInput").ap()
    d_out = nc.dram_tensor("outT", [B, D, T], FP16, kind="ExternalOutput").ap()

    with tile.TileContext(nc) as tc:
        with (
            tc.tile_pool(name="persist", bufs=1) as persist,
            tc.tile_pool(name="stream", bufs=6) as stream,
            tc.tile_pool(name="ee", bufs=5) as epool,
            tc.tile_pool(name="oc", bufs=3) as ocpool,
            tc.tile_pool(name="nrm", bufs=3) as nrm,
            tc.tile_pool(name="ps", bufs=4, space="PSUM") as ps,
            tc.tile_pool(name="opsum", bufs=4, space="PSUM") as ops,
        ):
            # ---- constants ----
            id8 = persist.tile([128, 128], FP8, tag="id8", name="id8")
            nc.scalar.dma_start(out=id8[:], in_=d_id8[:])
            kpad = persist.tile([128, B, NKC], F32, tag="kpad", name="kpad")
            nc.scalar.dma_start(out=kpad[:], in_=d_kp[:])
            ones = persist.tile([128, NKC], F32, tag="ones", name="ones")
            nc.vector.memset(ones[:], 1.0)
            ones16 = persist.tile([128, DK], FP16, tag="ones16", name="ones16")
            nc.vector.memset(ones16[:], 1.0)

            w_sb = {}
            for nm, dten in (("q", d_wq), ("k", d_wk), ("v", d_wv)):
                w = persist.tile([128, NDC, 128], FP16, tag=f"w{nm}",
                                 name=f"w{nm}")
                nc.scalar.dma_start(
                    out=w[:], in_=dten.rearrange("(a p) m -> p a m", p=128))
                w_sb[nm] = w
            # stacked Wo: rows = (h*DK + d), cols = D outputs
            wo2 = persist.tile([128, NDC, 128], FP16, tag="wo2", name="wo2")
            nc.scalar.dma_start(
                out=wo2[:], in_=d_wo.rearrange("p (a m) -> p a m", m=128))

            qt_sb, kt_sb, at2 = {}, {}, {}
            vaug = {}
            for b in range(B):
                kc = nkcb[b] * 128
                qt_sb[b] = persist.tile([128, T], FP16, tag=f"qt{b}",
                                        name=f"qt{b}")
                kt_sb[b] = persist.tile([128, kc], FP16, tag=f"kt{b}",
                                        name=f"kt{b}")
                at2[b] = persist.tile([128, T], FP16, tag=f"at{b}",
                                      name=f"at{b}")
                for h in range(HPC):
                    va = persist.tile([128, nkcb[b] * 80], FP16,
                                      tag=f"va{b}{h}", name=f"va{b}{h}")
                    va_c = va[:].rearrange("p (c u) -> p c u", u=80)
                    nc.vector.tensor_copy(va_c[:, :, 64], ones[:, 0:nkcb[b]])
                    vaug[(b, h)] = va

            # resident relpos tiles, shared across batches
            rp_sb = {}
            for h in range(HPC):
                for ck in range(nkc_max):
                    off = (ck // 4) * QG
                    rp_sb[(h, ck)] = persist.tile(
                        [128, T - off], FP8, tag=f"rp{h}_{ck}",
                        name=f"rp{h}_{ck}")

            # ---- projection helpers ----
            def load_x(dten, b, dk, eng, cols, tag, bufs):
                t = stream.tile([128, cols], FP16, tag=tag, bufs=bufs,
                                name=f"x{tag}{b}{dk}")
                eng.dma_start(out=t[:],
                              in_=dten[b, dk * 128:(dk + 1) * 128, 0:cols])
                return t

            def proj_qk(nm, b, xts, dst, cols):
                # slab-ordered: each 512-col slab finishes (PSUM freed,
                # dst slab copied) before the next begins
                for cc in range((cols + QG - 1) // QG):
                    w = min(QG, cols - cc * QG)
                    acc = ps.tile([128, QG], F32, tag="ps", name="ps")
                    for dk in range(NDC):
                        nc.tensor.matmul(
                            acc[:, 0:w], w_sb[nm][:, dk, :],
                            xts[dk][:, cc * QG:cc * QG + w],
                            start=(dk == 0), stop=(dk == NDC - 1))
                    nc.vector.tensor_copy(
                        dst[b][:, cc * QG:cc * QG + w], acc[:, 0:w])

            def proj_v(b, vts):
                for tb in range(nkcb[b]):
                    ts_ = slice(tb * 128, (tb + 1) * 128)
                    acc = ps.tile([128, 128], F32, tag="ps", name="psv")
                    for dk in range(NDC):
                        nc.tensor.matmul(
                            acc[:], vts[dk][:, ts_], w_sb["v"][:, dk, :],
                            start=(dk == 0), stop=(dk == NDC - 1))
                    for h in range(HPC):
                        nc.vector.tensor_copy(
                            vaug[(b, h)][:, tb * 80:tb * 80 + DK],
                            acc[:, h * DK:(h + 1) * DK])

            # ---- phase 1: batch 0 projections; loads split across rings ----
            engs = (nc.scalar, nc.sync)
            kc0, kc1 = nkcb[0] * 128, nkcb[1] * 128
            xq0 = {dk: load_x(d_qT, 0, dk, engs[dk % 2], T, "xq", 9)
                   for dk in range(NDC)}
            vch0 = {dk: load_x(d_vT, 0, dk, engs[dk % 2], kc0, "xv", 8)
                    for dk in range(NDC)}
            proj_qk("q", 0, xq0, qt_sb, T)
            xk0 = {dk: load_x(d_kT, 0, dk, engs[dk % 2], kc0, "xk", 8)
                   for dk in range(NDC)}
            proj_v(0, vch0)
            proj_qk("k", 0, xk0, kt_sb, kc0)

            # ---- phase 2: attention ----
            pend_ops, pend_dve = {}, {}
            pend_pe = []          # (unit, qg) whose norm_pe is still owed
            oproj_q = []          # (b, db, qg) out-proj pieces ready to emit
            pe_done = {}          # (b, qg) -> count of heads norm_pe'd
            n_oc = [0]

            def norm_dve(u, qg):
                o = nrm.tile([DK + 1, QG], F32, tag="oc2", name="oc2")
                nc.vector.tensor_copy(o[:], pend_ops[u][qg][:])
                rc = nrm.tile([DK + 1, QG], F32, tag="rc", name="rc")
                nc.vector.reciprocal_approx_fast(out=rc[:], in_=o[:])
                rch = nrm.tile([DK + 1, QG], FP16, tag="rch", name="rch")
                nc.vector.tensor_copy(rch[:], rc[:])
                pend_dve[(u, qg)] = (o, rch)

            def norm_pe(u, qg):
                b, h = u
                qs = slice(qg * QG, (qg + 1) * QG)
                o, rch = pend_dve.pop((u, qg))
                rb = ops.tile([DK, QG], F32, tag="ops", name="rb")
                nc.tensor.matmul(
                    rb[:], ones16[DK:DK + 1, :], rch[DK:DK + 1, :],
                    start=True, stop=True)
                if h == 0:
                    nc.vector.tensor_mul(at2[b][0:DK, qs], o[0:DK, :], rb[:])
                else:
                    tmp = nrm.tile([DK, QG], FP16, tag="tmp", name="tmp")
                    nc.vector.tensor_mul(tmp[:], o[0:DK, :], rb[:])
                    nc.sync.dma_start(out=at2[b][DK:128, qs], in_=tmp[:])
                k = (b, qg)
                pe_done[k] = pe_done.get(k, 0) + 1
                if pe_done[k] == HPC:
                    oproj_q.extend((b, db, qg) for db in range(NDC))

            def oproj_piece(b, db, qg):
                ds_ = slice(db * 128, (db + 1) * 128)
                qs = slice(qg * QG, (qg + 1) * QG)
                pp = ps.tile([128, QG], F32, tag="ps", name="pp")
                nc.tensor.matmul(pp[:], wo2[:, db, :], at2[b][:, qs],
                                 start=True, stop=True)
                oc = ocpool.tile([128, QG], FP16, tag="oc", name="oc")
                if n_oc[0] % 2 == 0:
                    nc.vector.tensor_copy(oc[:], pp[:])
                else:
                    nc.scalar.copy(oc[:], pp[:])
                n_oc[0] += 1
                nc.sync.dma_start(out=d_out[b, ds_, qs], in_=oc[:])

            units = [(0, 0), (0, 1), (1, 1), (1, 0)]
            # remaining oproj-capable (unit, ck>=4) slots after each point
            slots_after = []
            total = 0
            for b, h in reversed(units):
                sl = []
                for ck in reversed(range(nkcb[b])):
                    sl.append(total)
                    if ck >= 4:
                        total += 1
                slots_after.append(list(reversed(sl)))
            slots_after.reverse()

            xq1 = vch1 = None
            for ui, u in enumerate(units):
                b, h = u
                nkc = nkcb[b]
                hs = slice(h * DK, (h + 1) * DK)
                if ui == 0:
                    # prefetch batch-1 q and v rows on the otherwise-idle
                    # GpSimd (SWDGE) queue so pool-slot waits don't block
                    # the Scalar queue's exp instructions.
                    xq1 = {dk: load_x(d_qT, 1, dk, nc.gpsimd, T, "xq", 9)
                           for dk in range(NDC)}
                    vch1 = {dk: load_x(d_vT, 1, dk, nc.gpsimd, kc1, "xv", 8)
                            for dk in range(NDC)}
                if ui == 1:
                    proj_qk("q", 1, xq1, qt_sb, T)
                    xk1 = {dk: load_x(d_kT, 1, dk, nc.gpsimd, kc1, "xk", 8)
                           for dk in range(NDC)}
                    proj_v(1, vch1)
                    proj_qk("k", 1, xk1, kt_sb, kc1)
                pend_ops[u] = [ops.tile([DK + 1, QG], F32, tag="ops",
                                        name="ops") for _ in range(NQG)]
                o_ps = pend_ops[u]
                for ck in range(nkc):
                    if ck >= 4 and pend_pe:
                        norm_pe(*pend_pe.pop(0))
                    if ck >= 4 and oproj_q:
                        nslot = max(1, slots_after[ui][ck])
                        npiece = min(3, max(1,
                                            -(-len(oproj_q) // nslot)))
                        for _ in range(npiece):
                            if oproj_q:
                                oproj_piece(*oproj_q.pop(0))
                    qg0 = ck // 4          # first valid q-group
                    off = qg0 * QG         # start col of rp tile
                    rp_t = rp_sb[(h, ck)]
                    if ui < 2:
                        nc.sync.dma_start(
                            out=rp_t[:],
                            in_=d_rp[h, ck * 128:(ck + 1) * 128, off:T])
                    s_ts, cos, ws = [], [], []
                    for qg in range(qg0, NQG):
                        co = max(0, ck * 128 - qg * QG)
                        w = QG - co
                        cos.append(co)
                        ws.append(w)
                        s_t = ps.tile([128, QG], F32, tag="ps", name="ps")
                        s_ts.append(s_t)
                        nc.tensor.matmul(
                            s_t[:, 0:w],
                            kt_sb[b][hs, ck * 128:(ck + 1) * 128],
                            qt_sb[b][hs, qg * QG + co:(qg + 1) * QG],
                            start=True, stop=False)
                    for i, qg in enumerate(range(qg0, NQG)):
                        rj = qg * QG + cos[i] - off
                        nc.tensor.matmul(
                            s_ts[i][:, 0:ws[i]], id8[:],
                            rp_t[:, rj:rj + ws[i]],
                            start=False, stop=True)
                    e_ts = []
                    for i, qg in enumerate(range(qg0, NQG)):
                        e_t = epool.tile([128, QG], FP16, tag="ee", name="ee")
                        e_ts.append(e_t)
                        nc.scalar.activation(
                            e_t[:, 0:ws[i]], s_ts[i][:, 0:ws[i]],
                            mybir.ActivationFunctionType.Exp,
                            bias=kpad[:, b, ck:ck + 1])
                    for i, qg in enumerate(range(qg0, NQG)):
                        nc.tensor.matmul(
                            o_ps[qg][:, cos[i]:QG],
                            vaug[(b, h)][:, ck * 80:ck * 80 + DK + 1],
                            e_ts[i][:, 0:ws[i]],
                            start=(ck == 0),
                            stop=(ck == min(4 * qg + 3, nkc - 1)))
                    for qg in range(qg0, NQG):
                        if ck == min(4 * qg + 3, nkc - 1):
                            norm_dve(u, qg)
                            pend_pe.append((u, qg))
                if ui < 2:
                    # rp chunks only the longer batch needs
                    for ck in range(nkc, nkc_max):
                        nc.sync.dma_start(
                            out=rp_sb[(h, ck)][:],
                            in_=d_rp[h, ck * 128:(ck + 1) * 128,
                                     (ck // 4) * QG:T])
            while pend_pe or oproj_q:
                if pend_pe:
                    norm_pe(*pend_pe.pop(0))
                while oproj_q:
                    oproj_piece(*oproj_q.pop(0))

    nc.compile()
    return nc


def _prep_host(q, k, v, key_pad_mask, attn_mask, relpos_bias, Wq, Wk, Wv, Wo):
    f32, f16 = np.float32, np.float16
    qT = np.asarray(q, f32).transpose(0, 2, 1).astype(f16)
    kT = np.asarray(k, f32).transpose(0, 2, 1).astype(f16)
    vT = np.asarray(v, f32).transpose(0, 2, 1).astype(f16)

    mask = np.asarray(key_pad_mask)
    kb = np.where(mask, NEG, f32(0)).astype(f32)  # [B,T]
    kpadT = np.ascontiguousarray(kb.reshape(B, NKC, 128).transpose(2, 0, 1))
    # active k-chunks: everything up to the last chunk with a valid key
    nkcb = []
    for b in range(B):
        valid = np.nonzero(~mask[b])[0]
        nkcb.append(int(valid.max()) // 128 + 1 if valid.size else 1)
    nkcb = tuple(nkcb)

    maskT = np.asarray(attn_mask).T  # [k, q], True = masked (k > q)
    rp = np.asarray(relpos_bias, f32)

    id8 = np.eye(128, dtype=ml_dtypes.float8_e4m3)

    Wq = np.asarray(Wq, f32) * f32(1.0 / np.sqrt(DK))
    Wk = np.asarray(Wk, f32)
    Wv = np.asarray(Wv, f32)
    Wo = np.asarray(Wo, f32)

    in_maps = []
    for c in range(NCORES):
        rows = slice(c * 128, (c + 1) * 128)
        h0 = 2 * c
        rpT = np.where(maskT[None], f32(-240.0),
                       rp[h0:h0 + 2].transpose(0, 2, 1)).astype(
                           ml_dtypes.float8_e4m3)
        woT = np.ascontiguousarray(
            np.concatenate([Wo[:, (2 * c + h) * DK:(2 * c + h + 1) * DK].T
                            for h in range(HPC)], axis=0)).astype(f16)
        in_maps.append({
            "qT": qT, "kT": kT, "vT": vT,
            "relposT": np.ascontiguousarray(rpT),
            "kpadT": kpadT,
            "wqT": np.ascontiguousarray(Wq[rows].T).astype(f16),
            "wkT": np.ascontiguousarray(Wk[rows].T).astype(f16),
            "wvT": np.ascontiguousarray(Wv[rows].T).astype(f16),
            "woT": woT,
            "id8": id8,
        })
    return in_maps, nkcb


def run(trace=False, tmpdir=None, **inputs):
    in_maps, nkcb = _prep_host(**inputs)
    if nkcb not in _CACHE:
        _CACHE[nkcb] = _build_program(nkcb)
    nc = _CACHE[nkcb]
    res = run_bass_kernel_spmd(nc, in_maps, core_ids=list(range(NCORES)),
                               trace=trace, tmpdir=tmpdir)
    acc = res.results[0]["outT"].astype(np.float64)
    for c in range(1, NCORES):
        acc += res.results[c]["outT"]
    out = np.ascontiguousarray(acc.transpose(0, 2, 1)).astype(np.float32)
    return out, res


def kernel(**inputs) -> np.ndarray:
    out, _ = run(trace=False, **inputs)
    return out
